# revision 32
# baseline (speedup 1.0000x reference)
"""Trainium2 Bass kernel for nn_Recommender_2 (moe_routing).

Pipeline per core (8 NeuronCores, one TRN2 chip):
  Phase 1 (data-parallel over batch, 128 rows/core):
    - indirect-DMA gather of ratings embeddings (table pre-cast to bf16)
    - x^T tiles via PE transpose + DVE copy; the 16MB of expert
      weights stream in at one DMA per scan step, held back by a
      step-paced guard chain so they never contend with the scan's
      gathers/weights for HBM
    - 2-layer LSTM scan (T=50) in transposed [gate, batch] layout, gate
      order host-permuted to (g,i,f,o): per cell 3 ACT calls
      (tanh(g), sigmoid(i,f,o), tanh(c')) + DVE ops on bf16
      post-activation values, with the f*c product offloaded to the
      (otherwise idle) GpSimd engine once the embedding gathers drain.
      Layer-1 runs one step behind layer-0 so the two cells' engine work
      interleaves; layer-0 input MMs pipelined one step ahead.
    - ratings MLP -> r part of z^T
  Exchange: single AllGather of z^T (u+r) into shared DRAM.
  Phase 2 (expert-parallel, 8 experts/core over full batch):
    - batch blocks processed in rotated order: the core's own block
      first, straight from the local z^T tile — overlapping the
      collective's ~11.5us trigger delay + ~28us transfer; remote blocks
      fetched by indirect DMA from the gathered Z using a per-core index
      tensor. Host un-rotates the output blocks.
    - he = z @ W1_e accumulated in PSUM (bf16 weights, native H=1536, no
      padding); reduction = one DVE scalar_tensor_tensor
      (he max 0)*w2 with accum_out per expert, hidden under the PE
      stream of the next expert's GEMM. Per-slot bias add + output DMA.
Host reassembles [1024, 64] from per-core [1024, 8] outputs.
"""
import numpy as np
import ml_dtypes

import concourse.bacc as bacc
import concourse.bass as bass
import concourse.mybir as mybir
import concourse.tile as tile
from concourse.bass_utils import run_bass_kernel_spmd
from concourse.masks import make_identity

P = 128
NCORES = 8
B, T = 1024, 50
RV, RD, RH = 100000, 128, 256
R_OUT = 256
UV, UD, UDATA, U_OUT = 50000, 64, 32, 128
E, EIN, EH = 64, 384, 1536
EPC = E // NCORES
BL = B // NCORES
UIN = UD + UDATA
UH = 192
RHID = 512
NG = 8
T2_GP = 12          # first step whose f*c product runs on GpSimd

F32 = mybir.dt.float32
BF16 = mybir.dt.bfloat16
I32 = mybir.dt.int32
AF = mybir.ActivationFunctionType
ALU = mybir.AluOpType

_cache = {}


def _prep(inputs):
    f = lambda k: np.asarray(inputs[k], dtype=np.float32)
    bf = lambda a: np.ascontiguousarray(a, dtype=ml_dtypes.bfloat16)
    f32c = lambda a: np.ascontiguousarray(a, dtype=np.float32)

    # gate permutation i,f,g,o -> g,i,f,o  (g,i in PSUM bank0; tanh(g) and
    # sigmoid(i,f,o) are then 1+1 contiguous ACT calls)
    perm = np.r_[512:768, 0:256, 256:512, 768:1024]

    shared = {}
    lstm_bias = False
    for l in range(2):
        wih = f(f"lstm_W_ih_{l}")[perm]
        whh = f(f"lstm_W_hh_{l}")[perm]
        bias = (f(f"lstm_b_ih_{l}") + f(f"lstm_b_hh_{l}"))[perm]
        shared[f"wih{l}"] = bf(wih.T)            # [in, 1024]
        shared[f"whh{l}"] = bf(whh.T)            # [256, 1024]
        shared[f"lb{l}"] = f32c(bias.reshape(NG, P).T)
        lstm_bias = lstm_bias or bool(np.any(bias))

    shared["rw1"] = bf(f("r_W1"))
    shared["rw2"] = bf(f("r_W2"))
    shared["uw1"] = bf(f("u_W1"))
    shared["uw2"] = bf(f("u_W2"))
    mlpb = np.zeros((P, 9), np.float32)
    mlpb[:, 0:4] = f("r_b1").reshape(4, P).T
    mlpb[:, 4:6] = f("r_b2").reshape(2, P).T
    ub1 = f("u_b1")
    mlpb[:, 6] = ub1[0:128]
    mlpb[0:64, 7] = ub1[128:192]
    mlpb[:, 8] = f("u_b2")
    shared["mlpb"] = mlpb

    shared["remb"] = bf(f("ratings_emb"))        # bf16 table
    shared["uemb"] = f32c(f("user_emb"))

    w1 = f("exp_W1")                             # [64, 384, 1536]
    w2 = f("exp_W2").reshape(E, EH)              # [64, 1536]
    b1 = f("exp_b1")                             # [64, 1536]
    b2 = f("exp_b2").reshape(E)
    exp_b1_nz = bool(np.any(b1))

    ridx = np.asarray(inputs["ratings_tensor"]).astype(np.int32)
    uids = np.asarray(inputs["user_ids"]).astype(np.int32)
    udata = f("user_data")

    in_maps = []
    for c in range(NCORES):
        m = dict(shared)
        sl = slice(c * BL, (c + 1) * BL)
        m["ridx"] = np.ascontiguousarray(ridx[sl])
        m["uid"] = np.ascontiguousarray(uids[sl].reshape(BL, 1))
        m["udata"] = f32c(udata[sl])
        es = slice(c * EPC, (c + 1) * EPC)
        m["expw"] = bf(w1[es])                   # [8, 384, 1536]
        m["w2r"] = f32c(np.broadcast_to(
            w2[es][:, None, :], (EPC, P, EH)))
        m["b1p"] = bf(b1[es])                    # [8, 1536]
        # slot-major b2: column j*EPC+e is expert c*EPC+e for every slot j
        b2bc = np.zeros((P, E), np.float32)
        for j in range(NCORES):
            for e in range(EPC):
                b2bc[:, j * EPC + e] = b2[c * EPC + e]
        m["b2bc"] = b2bc
        # rotated z row indices: slot j reads batch block (c+j)%8
        zsel = np.zeros((P, NCORES), np.int32)
        for j in range(NCORES):
            zsel[:, j] = ((c + j) % NCORES) * P + np.arange(P)
        m["zsel"] = zsel
        in_maps.append(m)

    bp = dict(lstm_bias=lstm_bias, exp_b1=exp_b1_nz, exp_b2=bool(np.any(b2)),
              r_bias=bool(np.any(f("r_b1")) or np.any(f("r_b2"))))
    return in_maps, bp


def _build(bp, sim_single=False):
    NH = [(s, s + 512) for s in range(0, EH, 512)]

    nc = bacc.Bacc("TRN2", target_bir_lowering=False)
    d_ridx = nc.dram_tensor("ridx", [BL, T], I32, kind="ExternalInput")
    d_uid = nc.dram_tensor("uid", [BL, 1], I32, kind="ExternalInput")
    d_udata = nc.dram_tensor("udata", [BL, UDATA], F32, kind="ExternalInput")
    d_remb = nc.dram_tensor("remb", [RV, RD], BF16, kind="ExternalInput")
    d_uemb = nc.dram_tensor("uemb", [UV, UD], F32, kind="ExternalInput")
    d_wih = [nc.dram_tensor(f"wih{l}", [RD if l == 0 else RH, 4 * RH], BF16,
                            kind="ExternalInput") for l in range(2)]
    d_whh = [nc.dram_tensor(f"whh{l}", [RH, 4 * RH], BF16, kind="ExternalInput")
             for l in range(2)]
    d_lb = [nc.dram_tensor(f"lb{l}", [P, NG], F32, kind="ExternalInput")
            for l in range(2)]
    d_rw1 = nc.dram_tensor("rw1", [RH, RHID], BF16, kind="ExternalInput")
    d_rw2 = nc.dram_tensor("rw2", [RHID, R_OUT], BF16, kind="ExternalInput")
    d_uw1 = nc.dram_tensor("uw1", [UIN, UH], BF16, kind="ExternalInput")
    d_uw2 = nc.dram_tensor("uw2", [UH, U_OUT], BF16, kind="ExternalInput")
    d_mlpb = nc.dram_tensor("mlpb", [P, 9], F32, kind="ExternalInput")
    d_expw = nc.dram_tensor("expw", [EPC, EIN, EH], BF16, kind="ExternalInput")
    d_w2r = nc.dram_tensor("w2r", [EPC, P, EH], F32, kind="ExternalInput")
    d_b1p = nc.dram_tensor("b1p", [EPC, EH], BF16, kind="ExternalInput")
    d_b2bc = nc.dram_tensor("b2bc", [P, E], F32, kind="ExternalInput")
    d_zsel = nc.dram_tensor("zsel", [P, NCORES], I32, kind="ExternalInput")
    d_out = nc.dram_tensor("out", [B, EPC], F32, kind="ExternalOutput")

    with tile.TileContext(nc) as tc:
        with (
            tc.tile_pool(name="sb", bufs=1) as sb,
            tc.tile_pool(name="dr", bufs=1, space="DRAM") as dr,
        ):
            # ---- latency-critical loads first (indices feed the gathers) ----
            ridx_t = sb.tile([BL, T], I32)
            nc.sync.dma_start(out=ridx_t[:], in_=d_ridx[:])
            uid_t = sb.tile([BL, 1], I32)
            nc.sync.dma_start(out=uid_t[:], in_=d_uid[:])
            ident = sb.tile([P, P], F32)
            make_identity(nc, ident[:])          # gpsimd: before the gathers
            identb = sb.tile([P, P], BF16)
            make_identity(nc, identb[:])
            Uin = sb.tile([P, UIN], F32)
            X = sb.tile([P, T, RD], BF16)
            for t in range(6):
                nc.gpsimd.indirect_dma_start(
                    out=X[:, t, :], out_offset=None, in_=d_remb[:],
                    in_offset=bass.IndirectOffsetOnAxis(ap=ridx_t[:, t:t + 1], axis=0))
            nc.gpsimd.indirect_dma_start(
                out=Uin[:, 0:UD], out_offset=None, in_=d_uemb[:],
                in_offset=bass.IndirectOffsetOnAxis(ap=uid_t[:, 0:1], axis=0))
            nc.sync.dma_start(out=Uin[:, UD:UIN], in_=d_udata[:])
            for t in range(6, T):
                nc.gpsimd.indirect_dma_start(
                    out=X[:, t, :], out_offset=None, in_=d_remb[:],
                    in_offset=bass.IndirectOffsetOnAxis(ap=ridx_t[:, t:t + 1], axis=0))

            # ---- weights needed before/during the scan (Act HWDGE queue:
            # parallel to the sync queue's transposes and gpsimd gathers) ----
            uw1_t = sb.tile([UIN, UH], BF16)
            nc.sync.dma_start(out=uw1_t[:], in_=d_uw1[:])
            uw2a = sb.tile([P, U_OUT], BF16)
            nc.sync.dma_start(out=uw2a[:], in_=d_uw2[0:P, :])
            uw2b = sb.tile([UH - P, U_OUT], BF16)
            nc.sync.dma_start(out=uw2b[:], in_=d_uw2[P:UH, :])
            mlpb_t = sb.tile([P, 9], F32)
            nc.sync.dma_start(out=mlpb_t[:], in_=d_mlpb[:])
            wih_t = []
            whh_t = []
            for l in range(2):
                kin = RD if l == 0 else RH
                wt = []
                for kc in range(kin // P):
                    tl = sb.tile([P, 4 * RH], BF16, tag=f"wih{l}_{kc}")
                    nc.scalar.dma_start(out=tl[:], in_=d_wih[l][kc * P:(kc + 1) * P, :])
                    wt.append(tl)
                wih_t.append(wt)
                ht = []
                for kc in range(2):
                    tl = sb.tile([P, 4 * RH], BF16, tag=f"whh{l}_{kc}")
                    nc.scalar.dma_start(out=tl[:], in_=d_whh[l][kc * P:(kc + 1) * P, :])
                    ht.append(tl)
                whh_t.append(ht)
            lb_t = []
            if bp["lstm_bias"]:
                for l in range(2):
                    tl = sb.tile([P, NG], F32, tag=f"lb{l}")
                    nc.scalar.dma_start(out=tl[:], in_=d_lb[l][:])
                    lb_t.append(tl)
            if bp["exp_b1"]:
                ones1 = sb.tile([1, P], BF16)
                nc.gpsimd.memset(ones1[:], 1.0)

            # tiles loaded later (issued after the scan loop)
            rw1_t = [sb.tile([P, RHID], BF16, name=f"rw1_{kc}", tag=f"rw1_{kc}")
                     for kc in range(2)]
            rw2_t = [sb.tile([P, R_OUT], BF16, name=f"rw2_{kc}", tag=f"rw2_{kc}")
                     for kc in range(4)]
            b2bc_t = sb.tile([P, E], F32)
            zsel_t = sb.tile([P, NCORES], I32)
            w1e_t = [sb.tile([P, EIN // P, EH], BF16, name=f"w1e{e}", tag=f"w1e{e}")
                     for e in range(EPC)]
            w2r_t = [sb.tile([P, EH], F32, name=f"w2r{e}", tag=f"w2r{e}")
                     for e in range(EPC)]
            if bp["exp_b1"]:
                b1p_t = sb.tile([EPC, EH], BF16)
                nc.sync.dma_start(out=b1p_t[:], in_=d_b1p[:])
            for kc in range(2):
                nc.sync.dma_start(out=rw1_t[kc][:],
                                  in_=d_rw1[kc * P:(kc + 1) * P, :])
            for kc in range(4):
                nc.sync.dma_start(out=rw2_t[kc][:],
                                  in_=d_rw2[kc * P:(kc + 1) * P, :])
            nc.sync.dma_start(out=b2bc_t[:], in_=d_b2bc[:])
            nc.sync.dma_start(out=zsel_t[:], in_=d_zsel[:])
            # step-paced guard chain: tick[k] is written by a DVE copy at
            # step 8+k; the sync-ring guard read of it holds big weight DMA
            # k back until then (1 transfer/step instead of one 16MB burst)
            NSPREAD = 32
            tick = sb.tile([1, NSPREAD], F32)
            tickdr = dr.tile([1, NSPREAD], F32)
            big_dmas = []
            for e in range(EPC):
                for i in range(EIN // P):
                    big_dmas.append((w1e_t[e][:, i, :],
                                     d_expw[e, i * P:(i + 1) * P, :]))
            for e in range(EPC):
                big_dmas.append((w2r_t[e][:], d_w2r[e]))

            # warm both ACT function tables off the critical chain
            warm = sb.tile([1, 4], F32)
            nc.gpsimd.memset(warm[:], 0.0)
            nc.scalar.activation(warm[0:1, 0:2], warm[0:1, 2:4], AF.Tanh)
            nc.scalar.activation(warm[0:1, 0:2], warm[0:1, 2:4], AF.Sigmoid)

            zT = sb.tile([P, EIN], BF16)
            zdr = dr.tile([P, EIN], BF16)
            Zall = dr.tile([NCORES * P, EIN], BF16, addr_space="Shared")

            with (
                tc.tile_pool(name="ptm", bufs=2, space="PSUM") as ptm,
                tc.tile_pool(name="pXG", bufs=1, space="PSUM") as pXG,
                tc.tile_pool(name="pG1", bufs=1, space="PSUM") as pG1,
            ):
                # ---- user MLP: runs mid-scan (t==30); its output is
                # only needed by the z AllGather after the scan ----
                UinT = sb.tile([UIN, P], BF16)
                U1T = sb.tile([P, 2 * P], BF16)

                def u_mlp():
                    tru = ptm.tile([P, P], F32, tag="tm")
                    nc.tensor.transpose(out=tru[0:UIN, :], in_=Uin[:, :],
                                        identity=ident[:])
                    nc.vector.tensor_copy(out=UinT[:], in_=tru[0:UIN, :])
                    u1ps = ptm.tile([P, 2 * P], F32, tag="tm")
                    nc.tensor.matmul(out=u1ps[:, 0:P], lhsT=uw1_t[:, 0:P],
                                     rhs=UinT[:], start=True, stop=True)
                    nc.tensor.matmul(out=u1ps[0:UH - P, P:2 * P],
                                     lhsT=uw1_t[:, P:UH],
                                     rhs=UinT[:], start=True, stop=True)
                    nc.scalar.activation(U1T[:, 0:P], u1ps[:, 0:P], AF.Relu,
                                         bias=mlpb_t[:, 6:7])
                    nc.scalar.activation(U1T[0:UH - P, P:2 * P],
                                         u1ps[0:UH - P, P:2 * P],
                                         AF.Relu, bias=mlpb_t[0:UH - P, 7:8])
                    u2ps = ptm.tile([P, P], F32, tag="tm")
                    nc.tensor.matmul(out=u2ps[:], lhsT=uw2a[:], rhs=U1T[:, 0:P],
                                     start=True, stop=False)
                    nc.tensor.matmul(out=u2ps[:], lhsT=uw2b[:],
                                     rhs=U1T[0:UH - P, P:2 * P],
                                     start=False, stop=True)
                    nc.scalar.activation(zT[:, 0:P], u2ps[:], AF.Identity,
                                         bias=mlpb_t[:, 8:9])

                # x^T via PE transpose + DVE copy (a DMA-transpose variant
                # false-serializes against the gathers via the shared
                # semaphore pool and stalls the scan)
                XT = sb.tile([P, T, RD], BF16)

                def transpose_x(t):
                    tr = ptm.tile([P, P], BF16, name="tr", tag="tm")
                    nc.tensor.transpose(out=tr[:], in_=X[:, t, :],
                                        identity=identb[:])
                    nc.vector.tensor_copy(out=XT[:, t, :], in_=tr[:])

                for t in range(3):
                    transpose_x(t)

                # ---- LSTM scan ----
                XG = [pXG.tile([P, 4 * RH], F32, name="XG0"),
                      pXG.tile([P, 4 * RH], F32, name="XG1")]
                G1 = pG1.tile([P, 4 * RH], F32, name="G1")
                # post-activation gates [tanh_g | sig_i | sig_f | sig_o] bf16
                SG = [sb.tile([P, 4 * P * 2], BF16, name=f"SG{l}") for l in range(2)]
                CS = [sb.tile([P, RH], F32, name=f"C{l}") for l in range(2)]
                T1 = [sb.tile([P, RH], BF16, name=f"T1_{l}") for l in range(2)]
                T2 = [sb.tile([P, RH], F32, name=f"T2_{l}") for l in range(2)]
                TC = [sb.tile([P, RH], BF16, name=f"TC{l}") for l in range(2)]
                # h0 double-buffered: layer-1 (delayed one step) still needs
                # h0(t-1) after cell(0,t) has produced h0(t)
                hT0 = [sb.tile([P, RH], BF16, name=f"hT0_{j}") for j in range(2)]
                hT1 = sb.tile([P, RH], BF16, name="hT1")

                def cell(l, t, g, h):
                    sg, cs, t1, t2, tcl = SG[l], CS[l], T1[l], T2[l], TC[l]
                    if bp["lstm_bias"]:
                        # per-gate-block bias requires per-jg ACT calls
                        for jg in (0, 1):
                            nc.scalar.activation(sg[:, jg * P:(jg + 1) * P],
                                                 g[:, jg * P:(jg + 1) * P],
                                                 AF.Tanh,
                                                 bias=lb_t[l][:, jg:jg + 1])
                        for jg in (2, 3, 4, 5, 6, 7):
                            nc.scalar.activation(sg[:, jg * P:(jg + 1) * P],
                                                 g[:, jg * P:(jg + 1) * P],
                                                 AF.Sigmoid,
                                                 bias=lb_t[l][:, jg:jg + 1])
                    else:
                        nc.scalar.activation(sg[:, 0:256], g[:, 0:256], AF.Tanh)
                        # i,f first (feeds t1/t2), o split off: it only gates
                        # the final h-mult and stays off the c' chain
                        nc.scalar.activation(sg[:, 256:768], g[:, 256:768],
                                             AF.Sigmoid)
                        nc.scalar.activation(sg[:, 768:1024], g[:, 768:1024],
                                             AF.Sigmoid)
                    if t == 0:
                        nc.vector.tensor_tensor(out=cs[:, :], in0=sg[:, 256:512],
                                                in1=sg[:, 0:256], op=ALU.mult)
                    else:
                        # f*c on GpSimd (idle once the gathers drain) takes it
                        # off both the DVE queue and the critical chain
                        if t >= T2_GP and not bp["lstm_bias"]:
                            nc.gpsimd.tensor_tensor(out=t2[:, :],
                                                    in0=sg[:, 512:768],
                                                    in1=cs[:, :], op=ALU.mult)
                        else:
                            nc.vector.tensor_tensor(out=t2[:, :],
                                                    in0=sg[:, 512:768],
                                                    in1=cs[:, :], op=ALU.mult)
                        nc.vector.tensor_tensor(out=t1[:, :], in0=sg[:, 256:512],
                                                in1=sg[:, 0:256], op=ALU.mult)
                        nc.vector.tensor_tensor(out=cs[:, :], in0=t1[:, :],
                                                in1=t2[:, :], op=ALU.add)
                    nc.scalar.activation(tcl[:, :], cs[:, :], AF.Tanh)
                    nc.vector.tensor_tensor(out=h[:, :], in0=sg[:, 768:1024],
                                            in1=tcl[:, :], op=ALU.mult)

                # PSUM group discipline: start=True clears has_written for the
                # WHOLE bank (4 jg slices), so open each bank's group only on
                # its first slice and close it on the last.
                bank_first = lambda jg: jg % 4 == 0
                bank_last = lambda jg: jg % 4 == 3

                def l1_step(u):
                    """layer-1 MMs + cell for step u (issued one step late: at
                    issue time h0(u) is long ready -> no PE wait)."""
                    h0u = hT0[u % 2]
                    for jg in range(NG):
                        for kc in range(2):
                            nc.tensor.matmul(
                                out=G1[:, jg * P:(jg + 1) * P],
                                lhsT=wih_t[1][kc][:, jg * P:(jg + 1) * P],
                                rhs=h0u[:, kc * P:(kc + 1) * P],
                                start=(kc == 0 and bank_first(jg)),
                                stop=(u == 0 and kc == 1 and bank_last(jg)))
                    if u > 0:
                        for jg in range(NG):
                            for kc in range(2):
                                nc.tensor.matmul(
                                    out=G1[:, jg * P:(jg + 1) * P],
                                    lhsT=whh_t[1][kc][:, jg * P:(jg + 1) * P],
                                    rhs=hT1[:, kc * P:(kc + 1) * P],
                                    start=False, stop=(kc == 1 and bank_last(jg)))
                    cell(1, u, G1, hT1)

                # prologue: xg0 for t=0
                for jg in range(NG):
                    nc.tensor.matmul(out=XG[0][:, jg * P:(jg + 1) * P],
                                     lhsT=wih_t[0][0][:, jg * P:(jg + 1) * P],
                                     rhs=XT[:, 0, :], start=bank_first(jg),
                                     stop=bank_last(jg))

                for t in range(T):
                    Gx = XG[t % 2]
                    # L0 recurrent (the critical chain)
                    if t > 0:
                        for jg in range(NG):
                            for kc in range(2):
                                nc.tensor.matmul(
                                    out=Gx[:, jg * P:(jg + 1) * P],
                                    lhsT=whh_t[0][kc][:, jg * P:(jg + 1) * P],
                                    rhs=hT0[(t - 1) % 2][:, kc * P:(kc + 1) * P],
                                    start=False, stop=(kc == 1 and bank_last(jg)))
                    cell(0, t, Gx, hT0[t % 2])
                    # layer 1 for the previous step: all operands ready
                    if t > 0:
                        l1_step(t - 1)
                    # filler: xg0 for step t+1 (keeps PE warm, off-chain)
                    if t + 1 < T:
                        Gn = XG[(t + 1) % 2]
                        for jg in range(NG):
                            nc.tensor.matmul(
                                out=Gn[:, jg * P:(jg + 1) * P],
                                lhsT=wih_t[0][0][:, jg * P:(jg + 1) * P],
                                rhs=XT[:, t + 1, :], start=bank_first(jg), stop=False)
                    if t + 3 < T:
                        transpose_x(t + 3)
                    if 8 <= t < 8 + NSPREAD:
                        k = t - 8
                        # source is rewritten every step -> the copy (and the
                        # guarded DMA behind it) really waits for step t
                        nc.vector.tensor_copy(out=tick[0:1, k:k + 1],
                                              in_=TC[0][0:1, 0:1])
                        nc.sync.dma_start(out=tickdr[0:1, k:k + 1],
                                          in_=tick[0:1, k:k + 1])
                        dst, src_ = big_dmas[k]
                        nc.sync.dma_start(out=dst, in_=src_)
                    if t == 30:
                        u_mlp()
                l1_step(T - 1)

                # ---- ratings MLP ----
                r1ps = ptm.tile([P, RHID], F32, tag="tm")
                for mc in range(4):
                    for kc in range(2):
                        nc.tensor.matmul(
                            out=r1ps[:, mc * P:(mc + 1) * P],
                            lhsT=rw1_t[kc][:, mc * P:(mc + 1) * P],
                            rhs=hT1[:, kc * P:(kc + 1) * P],
                            start=(kc == 0), stop=(kc == 1))
                R1T = sb.tile([P, RHID], BF16)
                if bp["r_bias"]:
                    for mc in range(4):
                        nc.scalar.activation(R1T[:, mc * P:(mc + 1) * P],
                                             r1ps[:, mc * P:(mc + 1) * P],
                                             AF.Relu, bias=mlpb_t[:, mc:mc + 1])
                else:
                    nc.scalar.activation(R1T[:, :], r1ps[:, :], AF.Relu)
                r2ps = ptm.tile([P, R_OUT], F32, tag="tm")
                for mc in range(2):
                    for kc in range(4):
                        nc.tensor.matmul(
                            out=r2ps[:, mc * P:(mc + 1) * P],
                            lhsT=rw2_t[kc][:, mc * P:(mc + 1) * P],
                            rhs=R1T[:, kc * P:(kc + 1) * P],
                            start=(kc == 0), stop=(kc == 3))
                if bp["r_bias"]:
                    for mc in range(2):
                        nc.scalar.activation(zT[:, P + mc * P:P + (mc + 1) * P],
                                             r2ps[:, mc * P:(mc + 1) * P],
                                             AF.Identity,
                                             bias=mlpb_t[:, 4 + mc:5 + mc])
                else:
                    nc.scalar.activation(zT[:, P:P + R_OUT], r2ps[:, 0:R_OUT],
                                         AF.Identity)

            # ---- allgather z ----
            nc.sync.dma_start(out=zdr[:], in_=zT[:])
            if sim_single:
                nc.sync.dma_start(out=Zall[0:P, :], in_=zdr[:])
            else:
                nc.gpsimd.collective_compute(
                    "AllGather", ALU.bypass, ins=[zdr.opt()], outs=[Zall.opt()],
                    replica_groups=[list(range(NCORES))])

            # ---- experts: slot 0 = local batch block (no collective dep) ----
            with tc.tile_pool(name="phe", bufs=2, space="PSUM") as phe:
                nslots = 1 if sim_single else NCORES
                Zt = [zT]
                for j in range(1, nslots):
                    tl = sb.tile([P, EIN], BF16, tag=f"zt{j}")
                    nc.gpsimd.indirect_dma_start(
                        out=tl[:], out_offset=None, in_=Zall[:],
                        in_offset=bass.IndirectOffsetOnAxis(
                            ap=zsel_t[:, j:j + 1], axis=0))
                    Zt.append(tl)
                scrd = sb.tile([P, EH], BF16)
                souts = sb.tile([P, E], F32)
                outs = sb.tile([P, E], F32)
                for j in range(nslots):
                    for e in range(EPC):
                        he = phe.tile([P, EH], F32, name="he", tag="he")
                        for i in range(EIN // P):
                            first = i == 0
                            last = (i == EIN // P - 1) and not bp["exp_b1"]
                            for (n0, n1) in NH:
                                nc.tensor.matmul(
                                    out=he[:, n0:n1],
                                    lhsT=Zt[j][:, i * P:(i + 1) * P],
                                    rhs=w1e_t[e][:, i, n0:n1],
                                    start=first, stop=last)
                        if bp["exp_b1"]:
                            for (n0, n1) in NH:
                                nc.tensor.matmul(
                                    out=he[:, n0:n1], lhsT=ones1[:],
                                    rhs=b1p_t[e:e + 1, n0:n1],
                                    start=False, stop=True)
                        col = j * EPC + e
                        nc.vector.scalar_tensor_tensor(
                            out=scrd[:], in0=he[:], scalar=0.0,
                            in1=w2r_t[e][:], op0=ALU.max, op1=ALU.mult,
                            accum_out=souts[:, col:col + 1])
                    # per-slot bias add + output DMA (overlaps next slot)
                    sl = slice(j * EPC, (j + 1) * EPC)
                    if bp["exp_b2"]:
                        nc.vector.tensor_tensor(out=outs[:, sl], in0=souts[:, sl],
                                                in1=b2bc_t[:, sl], op=ALU.add)
                        fin = outs
                    else:
                        fin = souts
                    nc.sync.dma_start(out=d_out[j * P:(j + 1) * P, :],
                                      in_=fin[:, sl])
    nc.finalize()
    return nc


def _get_nc(bp, sim_single=False):
    key = (bp["lstm_bias"], bp["exp_b1"], bp["exp_b2"], bp["r_bias"], sim_single)
    if key not in _cache:
        _cache[key] = _build(bp, sim_single=sim_single)
    return _cache[key]


def run(inputs, trace=False):
    in_maps, bp = _prep(inputs)
    nc = _get_nc(bp)
    res = run_bass_kernel_spmd(nc, in_maps, core_ids=list(range(NCORES)),
                               trace=trace)
    out = np.empty((B, E), np.float32)
    for c in range(NCORES):
        o = np.asarray(res.results[c]["out"]).astype(np.float32)
        for j in range(NCORES):
            blk = (c + j) % NCORES
            out[blk * P:(blk + 1) * P, c * EPC:(c + 1) * EPC] = \
                o[j * P:(j + 1) * P]
    return out, res


def kernel(**inputs) -> np.ndarray:
    out, _ = run(inputs, trace=False)
    return out
